# revision 41
# baseline (speedup 1.0000x reference)
"""MinGRU forward on 8 TRN2 NeuronCores.

Math (linear-space reformulation of the reference's log-space Heinsen scan):
    hg = x @ W_hg.T                       # [B,S,2D]
    hidden, gate = split(hg)
    z = sigmoid(gate)
    c = sigmoid(-gate)                    # = 1 - z = exp(-softplus(gate))
    g = max(hidden + 0.5, sigmoid(hidden))  # == where(h>=0, h+0.5, sigmoid(h)) exactly
    u = z * g
    h[t] = c[t] * h[t-1] + u[t]           # convex combination -> bounded, stable
    out = h

The recurrence maps directly onto the DVE `tensor_tensor_scan` instruction
(state = data0*state + data1 along the free dim, fp32 internal state).

Sharding: 8 cores = 4 batches x 2 feature-halves (512 features each).
No cross-core communication: the scan is per-feature independent.
The host packs x and W into the exact per-partition SBUF image (transposed,
k-major, chunk-contiguous) so every chunk is one DMA with multi-KB
contiguous lines.  Inputs are bf16 (halves DMA traffic, enables fast
weight load, rel-err ~1.5e-2 vs the 2e-2 gate); matmuls accumulate fp32
in PSUM; everything downstream of PSUM is fp32.
"""

import numpy as np

B, S, D = 4, 4096, 1024
DH = D // 2          # features per core
N_CORES = 8
KC = 128             # contraction chunk
NKC = D // KC        # 8 k chunks
FC = 128             # feature chunk (psum partitions)
NFC = DH // FC       # 4 feature chunks

_CACHE = {}

# build-time knobs (A/B tuning)
CONFIG = {
    "in_dtype": "bf16",   # "bf16" | "f32r"
    "out_ring": "sync",   # "scalar" | "sync"
    "widths": [512, 512, 512, 512, 512, 512, 512, 384, 128],
    "xbufs": 3,
    "psbufs": 4,
    "ebufs": 3,
    "u_on_gpsimd": False,  # compute u = z*g on GpSimd instead of DVE
    # (measured: GpSimd tensor ops contend for SBUF ports and slow PE/DVE/ACT
    # by ~13-20% across the board — keep ALL tensor work off GpSimd)
    "split_last_scan": False,  # last chunk: 2 chained half-scans (extra DMAs cost
    # ~830ns serial issue each at the tail — net loss)
    # The framework's end-of-kernel epilogue zeroes the ENTIRE kernel
    # semaphore range one EVENT_SEMAPHORE per sem (~28ns each, ~7us for the
    # default [7,256) range) inside the measured window.  The kernel needs
    # far fewer sems (the Tile allocator recycles via mid-kernel range
    # clears), so narrowing the range shrinks that epilogue directly.
    # 160 exhausts the pool; 184 works but measures the same as 216 (the
    # per-engine zeroing blocks run in parallel), so keep the extra margin.
    "sem_range_stop": 216,  # None = framework default (256)
}


def _round_fp32r(a: np.ndarray) -> np.ndarray:
    """Round fp32 array to fp32r (11 explicit mantissa bits) with RNE."""
    u = np.ascontiguousarray(a, dtype=np.float32).view(np.uint32)
    r = (u + np.uint32(0x7FF) + ((u >> np.uint32(12)) & np.uint32(1))) & np.uint32(0xFFFFF000)
    return r.view(np.float32)


def _build():
    import concourse.bacc as bacc
    import concourse.bass as bass_mod
    import concourse.tile as tile
    import concourse.mybir as mybir

    if CONFIG["sem_range_stop"] is not None:
        stop = CONFIG["sem_range_stop"]
        bass_mod.get_kernel_semaphore_range = (
            lambda: range(bass_mod.get_walrus_max_sem_num(), stop)
        )

    f32 = mybir.dt.float32
    in_dt = mybir.dt.bfloat16 if CONFIG["in_dtype"] == "bf16" else mybir.dt.float32r
    AF = mybir.ActivationFunctionType
    OP = mybir.AluOpType

    nc = bacc.Bacc("TRN2")
    # Host pre-packs both inputs into the exact SBUF image the kernel wants:
    # per partition p, chunk sc occupies a contiguous [NKC * width] run
    # (k-major).  This makes every chunk ONE DMA with multi-KB contiguous
    # lines per partition — short lines (<2KB) tank HWDGE throughput.
    xQ = nc.dram_tensor("xQ", [KC, NKC * S], in_dt, kind="ExternalInput")
    # wQ: per partition, fc-major then k-major then 256 cols (128 hidden,
    # 128 gate).
    wQ = nc.dram_tensor("wQ", [KC, NFC * NKC * 2 * FC], in_dt, kind="ExternalInput")
    outT = nc.dram_tensor("outT", [DH, S], f32, kind="ExternalOutput")

    widths = CONFIG["widths"]
    assert sum(widths) == S
    WFC = NKC * 2 * FC  # weight elems per partition per fc (2048)

    with tile.TileContext(nc) as tc:
        with (
            tc.tile_pool(name="w", bufs=1) as wpool,
            tc.tile_pool(name="x", bufs=CONFIG["xbufs"]) as xpool,
            tc.tile_pool(name="ew", bufs=CONFIG["ebufs"]) as epool,
            tc.tile_pool(name="h", bufs=2) as hpool,
            tc.tile_pool(name="ps", bufs=CONFIG["psbufs"], space="PSUM") as pspool,
        ):
            # W rides the ACT HWDGE ring; x rides the SP ring.  The ramp is
            # DMA-critical: the very first descriptors on each ring are the
            # small slices the first matmuls need, so the PE starts early
            # and stays busy (warming HAM's clock-gate with real work).
            wts = []
            for fc in range(NFC):
                wtf = wpool.tile([KC, WFC], in_dt, tag=f"w{fc}")
                wts.append(wtf)

            # (HAM clock-gate warm-up via dummy matmuls was tried twice —
            # sparse tiny MMs don't trip the activity window, and dense ones
            # reshuffle the Tile scheduler's DMA issue order and starve the
            # early pipeline.  Both regressed; do not re-add.)

            # fc0 weights and x chunk 0 each arrive in four k-pair slices so
            # the first matmuls start (and keep going) as the data streams in
            # — paced PE consumption also warms the HAM clock-gate sooner.
            for q in range(4):
                a, b = q * WFC // 4, (q + 1) * WFC // 4
                nc.scalar.dma_start(wts[0][:, a:b], wQ[:, a:b])
            w0 = widths[0]
            xt0 = xpool.tile([KC, NKC * w0], in_dt, tag="xt")
            # k0 alone first: the critical path to matmul #1 is then 128KB on
            # each ring (w k0-pair on scalar, x k0 on sync) instead of 256KB
            for a, b in [(0, w0), (w0, 2 * w0), (2 * w0, 4 * w0), (4 * w0, 8 * w0)]:
                nc.sync.dma_start(xt0[:, a:b], xQ[:, a:b])
            for fc in range(1, NFC):
                nc.scalar.dma_start(wts[fc][:], wQ[:, fc * WFC:(fc + 1) * WFC])

            out_eng = nc.scalar if CONFIG["out_ring"] == "scalar" else nc.sync

            # One piece (scan + out-DMA granule) per chunk.  Pairing chunks
            # into wider pieces was measured neutral-to-worse: the PE, not
            # the DVE, is the binding engine, and wide pieces near the end
            # push large output transfers into the tail.
            pieces = [(sc,) for sc in range(len(widths))]

            hprev = [None] * NFC
            prev_pwidth = 0
            off = 0
            boff = 0  # element offset into xQ's per-partition run
            for pi, piece in enumerate(pieces):
                pwidth = sum(widths[s] for s in piece)
                poff_base = off
                cts = [None] * NFC
                uts = [None] * NFC
                for j, sc in enumerate(piece):
                    width = widths[sc]
                    if sc == 0:
                        xt = xt0
                    else:
                        xt = xpool.tile([KC, NKC * width], in_dt, tag="xt")
                        if sc == 1:
                            # still inside the ramp: split for earlier arrival
                            h_ = NKC * width // 2
                            nc.sync.dma_start(xt[:, 0:h_], xQ[:, boff:boff + h_])
                            nc.sync.dma_start(xt[:, h_:], xQ[:, boff + h_:boff + NKC * width])
                        else:
                            nc.sync.dma_start(xt[:], xQ[:, boff:boff + NKC * width])
                    poff = off - poff_base
                    for fc in range(NFC):
                        ph = pspool.tile([FC, width], f32, tag="ph")
                        pg = pspool.tile([FC, width], f32, tag="pg")
                        if sc == 0:
                            # ramp: alternate h/g per k so the PE consumes
                            # each k-pair DMA slice the moment it lands
                            for k in range(NKC):
                                nc.tensor.matmul(
                                    ph[:], wts[fc][:, k * 2 * FC:k * 2 * FC + FC],
                                    xt[:, k * width:(k + 1) * width],
                                    start=(k == 0), stop=(k == NKC - 1),
                                )
                                nc.tensor.matmul(
                                    pg[:], wts[fc][:, k * 2 * FC + FC:(k + 1) * 2 * FC],
                                    xt[:, k * width:(k + 1) * width],
                                    start=(k == 0), stop=(k == NKC - 1),
                                )
                        else:
                            for k in range(NKC):
                                nc.tensor.matmul(
                                    ph[:], wts[fc][:, k * 2 * FC:k * 2 * FC + FC],
                                    xt[:, k * width:(k + 1) * width],
                                    start=(k == 0), stop=(k == NKC - 1),
                                )
                            for k in range(NKC):
                                nc.tensor.matmul(
                                    pg[:], wts[fc][:, k * 2 * FC + FC:(k + 1) * 2 * FC],
                                    xt[:, k * width:(k + 1) * width],
                                    start=(k == 0), stop=(k == NKC - 1),
                                )
                        if j == 0:
                            cts[fc] = hpool.tile(
                                [FC, pwidth], f32, tag=f"c{fc}", name=f"c{fc}"
                            )
                            uts[fc] = hpool.tile(
                                [FC, pwidth], f32, tag=f"u{fc}", name=f"u{fc}"
                            )
                        ct, ut = cts[fc], uts[fc]
                        zt = epool.tile([FC, width], f32, tag="z")
                        st = epool.tile([FC, width], f32, tag="s")
                        gt = epool.tile([FC, width], f32, tag="g")
                        # s first: it heads the DVE critical chain (s->g->u->scan)
                        nc.scalar.activation(st[:], ph[:], AF.Sigmoid)
                        nc.scalar.activation(zt[:], pg[:], AF.Sigmoid)
                        # c = 1 - z on the DVE (fast 2x tensor_scalar) instead
                        # of a third ACT sigmoid: ACT is the tail serializer
                        nc.vector.tensor_scalar(
                            ct[:, poff:poff + width], zt[:], -1.0, 1.0,
                            op0=OP.mult, op1=OP.add,
                        )
                        # g = (hidden + 0.5) max sigmoid(hidden)
                        nc.vector.scalar_tensor_tensor(
                            gt[:], ph[:], 0.5, st[:], op0=OP.add, op1=OP.max
                        )
                        nc.vector.tensor_mul(ut[:, poff:poff + width], zt[:], gt[:])
                        if j == len(piece) - 1:
                            ht = hpool.tile([FC, pwidth], f32, tag=f"h{fc}")
                            init = (
                                0.0 if pi == 0
                                else hprev[fc][:, prev_pwidth - 1:prev_pwidth]
                            )
                            nc.vector.tensor_tensor_scan(
                                ht[:], ct[:], ut[:], init, op0=OP.mult, op1=OP.add
                            )
                            hprev[fc] = ht
                            # near the tail, out-DMA issue cost (~800ns serial
                            # per dma_start) dominates: spread across both rings
                            oe = out_eng
                            if pi >= len(pieces) - 2 and fc % 2 == 1:
                                oe = nc.scalar if out_eng is nc.sync else nc.sync
                            oe.dma_start(
                                outT[fc * FC:(fc + 1) * FC, poff_base:poff_base + pwidth],
                                ht[:],
                            )
                    off += width
                    boff += NKC * width
                prev_pwidth = pwidth

    nc.compile()
    return nc


def _to_in_dtype(a: np.ndarray) -> np.ndarray:
    if CONFIG["in_dtype"] == "bf16":
        import ml_dtypes
        return np.ascontiguousarray(a, dtype=np.float32).astype(ml_dtypes.bfloat16)
    return _round_fp32r(a)


def _prep_in_maps(x: np.ndarray, W_hg: np.ndarray):
    x = np.asarray(x, dtype=np.float32)
    W_hg = np.asarray(W_hg, dtype=np.float32)
    widths = CONFIG["widths"]

    # xQ [128, NKC*S]: per partition p, chunk sc holds xT[k*128+p, off:off+w]
    # k-major and contiguous, matching the SBUF tile image exactly.
    xQs = []
    for b in range(B):
        xT = np.ascontiguousarray(x[b].T)            # [D, S]
        xr = xT.reshape(NKC, KC, S)                   # [k, p, s]
        segs = []
        off = 0
        for w in widths:
            seg = xr[:, :, off:off + w]               # [k, p, w]
            segs.append(seg.transpose(1, 0, 2).reshape(KC, NKC * w))
            off += w
        xQs.append(_to_in_dtype(np.concatenate(segs, axis=1)))

    # wQ [128, NFC*NKC*256]: fc-major, then k, then 128 hidden + 128 gate cols
    wQs = []
    for c in range(2):
        wt = np.empty((D, NFC, 2 * FC), dtype=np.float32)
        for fc in range(NFC):
            rows_h = W_hg[c * DH + fc * FC:c * DH + (fc + 1) * FC]      # [FC, D]
            rows_g = W_hg[D + c * DH + fc * FC:D + c * DH + (fc + 1) * FC]
            wt[:, fc, 0:FC] = rows_h.T
            wt[:, fc, FC:2 * FC] = rows_g.T
        wr = wt.reshape(NKC, KC, NFC, 2 * FC)         # [k, p, fc, e]
        wq = wr.transpose(1, 2, 0, 3).reshape(KC, NFC * NKC * 2 * FC)
        wQs.append(_to_in_dtype(wq))
    return [{"xQ": xQs[core // 2], "wQ": wQs[core % 2]} for core in range(N_CORES)]


def _get_runner():
    """Build the Bass module once and cache a compiled jax callable for it.

    Mirrors bass2jax.run_bass_via_pjrt's multi-core path, but keeps the
    jitted/sharded executable so repeat kernel() calls skip re-tracing.
    """
    if "runner" in _CACHE:
        return _CACHE["runner"]

    import jax
    from jax.experimental.shard_map import shard_map
    from jax.sharding import Mesh, PartitionSpec
    from concourse import bass2jax

    if "nc" not in _CACHE:
        _CACHE["nc"] = _build()
    nc = _CACHE["nc"]
    bass2jax.install_neuronx_cc_hook()

    in_names = ["xQ", "wQ"]
    out_name = "outT"
    out_shape, out_dtype = (DH, S), np.float32
    partition_name = nc.partition_id_tensor.name if nc.partition_id_tensor else None

    def _body(xT, wT, zout):
        operands = [xT, wT, zout]
        if partition_name is not None:
            operands.append(bass2jax.partition_id_tensor())
        outs = bass2jax._bass_exec_p.bind(
            *operands,
            out_avals=(jax.core.ShapedArray(out_shape, out_dtype),),
            in_names=tuple(in_names + [out_name] + ([partition_name] if partition_name else [])),
            out_names=(out_name,),
            lowering_input_output_aliases=(),
            sim_require_finite=True,
            sim_require_nnan=True,
            nc=nc,
        )
        return tuple(outs)

    devices = jax.devices()[:N_CORES]
    mesh = Mesh(np.asarray(devices), ("core",))
    sharded = jax.jit(
        shard_map(
            _body, mesh=mesh,
            in_specs=(PartitionSpec("core"),) * 3,
            out_specs=(PartitionSpec("core"),),
            check_rep=False,
        ),
        donate_argnums=(2,),
        keep_unused=True,
    )

    def run(in_maps):
        concat_x = np.concatenate([m["xQ"] for m in in_maps], axis=0)
        concat_w = np.concatenate([m["wQ"] for m in in_maps], axis=0)
        zeros = np.zeros((N_CORES * DH, S), np.float32)
        (out_arr,) = sharded(concat_x, concat_w, zeros)
        return np.asarray(out_arr).reshape(N_CORES, DH, S)

    _CACHE["runner"] = run
    return run


def kernel(x: np.ndarray, W_hg: np.ndarray) -> np.ndarray:
    run = _get_runner()
    in_maps = _prep_in_maps(x, W_hg)
    outs = run(in_maps)

    out = np.empty((B, S, D), dtype=np.float32)
    for core in range(N_CORES):
        b, c = core // 2, core % 2
        out[b, :, c * DH:(c + 1) * DH] = outs[core].T
    return out


# revision 42
# speedup vs baseline: 1.0034x; 1.0034x over previous
"""MinGRU forward on 8 TRN2 NeuronCores.

Math (linear-space reformulation of the reference's log-space Heinsen scan):
    hg = x @ W_hg.T                       # [B,S,2D]
    hidden, gate = split(hg)
    z = sigmoid(gate)
    c = sigmoid(-gate)                    # = 1 - z = exp(-softplus(gate))
    g = max(hidden + 0.5, sigmoid(hidden))  # == where(h>=0, h+0.5, sigmoid(h)) exactly
    u = z * g
    h[t] = c[t] * h[t-1] + u[t]           # convex combination -> bounded, stable
    out = h

The recurrence maps directly onto the DVE `tensor_tensor_scan` instruction
(state = data0*state + data1 along the free dim, fp32 internal state).

Sharding: 8 cores = 4 batches x 2 feature-halves (512 features each).
No cross-core communication: the scan is per-feature independent.
The host packs x and W into the exact per-partition SBUF image (transposed,
k-major, chunk-contiguous) so every chunk is one DMA with multi-KB
contiguous lines.  Inputs are bf16 (halves DMA traffic, enables fast
weight load, rel-err ~1.5e-2 vs the 2e-2 gate); matmuls accumulate fp32
in PSUM; everything downstream of PSUM is fp32.
"""

import numpy as np

B, S, D = 4, 4096, 1024
DH = D // 2          # features per core
N_CORES = 8
KC = 128             # contraction chunk
NKC = D // KC        # 8 k chunks
FC = 128             # feature chunk (psum partitions)
NFC = DH // FC       # 4 feature chunks

_CACHE = {}

# build-time knobs (A/B tuning)
CONFIG = {
    "in_dtype": "bf16",   # "bf16" | "f32r"
    "out_ring": "sync",   # "scalar" | "sync"
    "widths": [512, 512, 512, 512, 512, 512, 512, 384, 128],
    "xbufs": 3,
    "psbufs": 4,
    "ebufs": 3,
    "u_on_gpsimd": False,  # compute u = z*g on GpSimd instead of DVE
    # (measured: GpSimd tensor ops contend for SBUF ports and slow PE/DVE/ACT
    # by ~13-20% across the board — keep ALL tensor work off GpSimd)
    "split_last_scan": False,  # last chunk: 2 chained half-scans (extra DMAs cost
    # ~830ns serial issue each at the tail — net loss)
    # The framework's end-of-kernel epilogue zeroes the ENTIRE kernel
    # semaphore range one EVENT_SEMAPHORE per sem (~28ns each, ~7us for the
    # default [7,256) range) inside the measured window.  The kernel needs
    # far fewer sems (the Tile allocator recycles via mid-kernel range
    # clears), so narrowing the range shrinks that epilogue directly.
    # 160 exhausts the pool; 184 works but measures the same as 216 (the
    # per-engine zeroing blocks run in parallel), so keep the extra margin.
    "sem_range_stop": 216,  # None = framework default (256)
}


def _round_fp32r(a: np.ndarray) -> np.ndarray:
    """Round fp32 array to fp32r (11 explicit mantissa bits) with RNE."""
    u = np.ascontiguousarray(a, dtype=np.float32).view(np.uint32)
    r = (u + np.uint32(0x7FF) + ((u >> np.uint32(12)) & np.uint32(1))) & np.uint32(0xFFFFF000)
    return r.view(np.float32)


def _build():
    import concourse.bacc as bacc
    import concourse.bass as bass_mod
    import concourse.tile as tile
    import concourse.mybir as mybir

    if CONFIG["sem_range_stop"] is not None:
        stop = CONFIG["sem_range_stop"]
        bass_mod.get_kernel_semaphore_range = (
            lambda: range(bass_mod.get_walrus_max_sem_num(), stop)
        )

    f32 = mybir.dt.float32
    in_dt = mybir.dt.bfloat16 if CONFIG["in_dtype"] == "bf16" else mybir.dt.float32r
    AF = mybir.ActivationFunctionType
    OP = mybir.AluOpType

    nc = bacc.Bacc("TRN2")
    # Host pre-packs both inputs into the exact SBUF image the kernel wants:
    # per partition p, chunk sc occupies a contiguous [NKC * width] run
    # (k-major).  This makes every chunk ONE DMA with multi-KB contiguous
    # lines per partition — short lines (<2KB) tank HWDGE throughput.
    xQ = nc.dram_tensor("xQ", [KC, NKC * S], in_dt, kind="ExternalInput")
    # wQ: per partition, fc-major then k-major then 256 cols (128 hidden,
    # 128 gate).
    wQ = nc.dram_tensor("wQ", [KC, NFC * NKC * 2 * FC], in_dt, kind="ExternalInput")
    outT = nc.dram_tensor("outT", [DH, S], f32, kind="ExternalOutput")

    widths = CONFIG["widths"]
    assert sum(widths) == S
    WFC = NKC * 2 * FC  # weight elems per partition per fc (2048)

    with tile.TileContext(nc) as tc:
        with (
            tc.tile_pool(name="w", bufs=1) as wpool,
            tc.tile_pool(name="x", bufs=CONFIG["xbufs"]) as xpool,
            tc.tile_pool(name="ew", bufs=CONFIG["ebufs"]) as epool,
            tc.tile_pool(name="h", bufs=2) as hpool,
            tc.tile_pool(name="ps", bufs=CONFIG["psbufs"], space="PSUM") as pspool,
        ):
            # W rides the ACT HWDGE ring; x rides the SP ring.  The ramp is
            # DMA-critical: the very first descriptors on each ring are the
            # small slices the first matmuls need, so the PE starts early
            # and stays busy (warming HAM's clock-gate with real work).
            wts = []
            for fc in range(NFC):
                wtf = wpool.tile([KC, WFC], in_dt, tag=f"w{fc}")
                wts.append(wtf)

            # (HAM clock-gate warm-up via dummy matmuls was tried twice —
            # sparse tiny MMs don't trip the activity window, and dense ones
            # reshuffle the Tile scheduler's DMA issue order and starve the
            # early pipeline.  Both regressed; do not re-add.)

            # fc0 weights and x chunk 0 each arrive in four k-pair slices so
            # the first matmuls start (and keep going) as the data streams in
            # — paced PE consumption also warms the HAM clock-gate sooner.
            for q in range(4):
                a, b = q * WFC // 4, (q + 1) * WFC // 4
                nc.scalar.dma_start(wts[0][:, a:b], wQ[:, a:b])
            w0 = widths[0]
            xt0 = xpool.tile([KC, NKC * w0], in_dt, tag="xt")
            # k-pair slices (NOT k0-alone-first: starting the PE earlier on a
            # sparser trickle delays the HAM busy-window trip and nets worse)
            for q in range(4):
                a, b = q * 2 * w0, (q + 1) * 2 * w0
                nc.sync.dma_start(xt0[:, a:b], xQ[:, a:b])
            for fc in range(1, NFC):
                nc.scalar.dma_start(wts[fc][:], wQ[:, fc * WFC:(fc + 1) * WFC])

            out_eng = nc.scalar if CONFIG["out_ring"] == "scalar" else nc.sync

            # One piece (scan + out-DMA granule) per chunk.  Pairing chunks
            # into wider pieces was measured neutral-to-worse: the PE, not
            # the DVE, is the binding engine, and wide pieces near the end
            # push large output transfers into the tail.
            pieces = [(sc,) for sc in range(len(widths))]

            hprev = [None] * NFC
            prev_pwidth = 0
            off = 0
            boff = 0  # element offset into xQ's per-partition run
            for pi, piece in enumerate(pieces):
                pwidth = sum(widths[s] for s in piece)
                poff_base = off
                cts = [None] * NFC
                uts = [None] * NFC
                for j, sc in enumerate(piece):
                    width = widths[sc]
                    if sc == 0:
                        xt = xt0
                    else:
                        xt = xpool.tile([KC, NKC * width], in_dt, tag="xt")
                        if sc == 1:
                            # still inside the ramp: split for earlier arrival
                            h_ = NKC * width // 2
                            nc.sync.dma_start(xt[:, 0:h_], xQ[:, boff:boff + h_])
                            nc.sync.dma_start(xt[:, h_:], xQ[:, boff + h_:boff + NKC * width])
                        else:
                            nc.sync.dma_start(xt[:], xQ[:, boff:boff + NKC * width])
                    poff = off - poff_base
                    for fc in range(NFC):
                        ph = pspool.tile([FC, width], f32, tag="ph")
                        pg = pspool.tile([FC, width], f32, tag="pg")
                        if sc == 0:
                            # ramp: alternate h/g per k so the PE consumes
                            # each k-pair DMA slice the moment it lands
                            for k in range(NKC):
                                nc.tensor.matmul(
                                    ph[:], wts[fc][:, k * 2 * FC:k * 2 * FC + FC],
                                    xt[:, k * width:(k + 1) * width],
                                    start=(k == 0), stop=(k == NKC - 1),
                                )
                                nc.tensor.matmul(
                                    pg[:], wts[fc][:, k * 2 * FC + FC:(k + 1) * 2 * FC],
                                    xt[:, k * width:(k + 1) * width],
                                    start=(k == 0), stop=(k == NKC - 1),
                                )
                        else:
                            for k in range(NKC):
                                nc.tensor.matmul(
                                    ph[:], wts[fc][:, k * 2 * FC:k * 2 * FC + FC],
                                    xt[:, k * width:(k + 1) * width],
                                    start=(k == 0), stop=(k == NKC - 1),
                                )
                            for k in range(NKC):
                                nc.tensor.matmul(
                                    pg[:], wts[fc][:, k * 2 * FC + FC:(k + 1) * 2 * FC],
                                    xt[:, k * width:(k + 1) * width],
                                    start=(k == 0), stop=(k == NKC - 1),
                                )
                        if j == 0:
                            cts[fc] = hpool.tile(
                                [FC, pwidth], f32, tag=f"c{fc}", name=f"c{fc}"
                            )
                            uts[fc] = hpool.tile(
                                [FC, pwidth], f32, tag=f"u{fc}", name=f"u{fc}"
                            )
                        ct, ut = cts[fc], uts[fc]
                        zt = epool.tile([FC, width], f32, tag="z")
                        st = epool.tile([FC, width], f32, tag="s")
                        gt = epool.tile([FC, width], f32, tag="g")
                        # s first: it heads the DVE critical chain (s->g->u->scan)
                        nc.scalar.activation(st[:], ph[:], AF.Sigmoid)
                        nc.scalar.activation(zt[:], pg[:], AF.Sigmoid)
                        # c = 1 - z on the DVE (fast 2x tensor_scalar) instead
                        # of a third ACT sigmoid: ACT is the tail serializer
                        nc.vector.tensor_scalar(
                            ct[:, poff:poff + width], zt[:], -1.0, 1.0,
                            op0=OP.mult, op1=OP.add,
                        )
                        # g = (hidden + 0.5) max sigmoid(hidden)
                        nc.vector.scalar_tensor_tensor(
                            gt[:], ph[:], 0.5, st[:], op0=OP.add, op1=OP.max
                        )
                        nc.vector.tensor_mul(ut[:, poff:poff + width], zt[:], gt[:])
                        if j == len(piece) - 1:
                            ht = hpool.tile([FC, pwidth], f32, tag=f"h{fc}")
                            init = (
                                0.0 if pi == 0
                                else hprev[fc][:, prev_pwidth - 1:prev_pwidth]
                            )
                            nc.vector.tensor_tensor_scan(
                                ht[:], ct[:], ut[:], init, op0=OP.mult, op1=OP.add
                            )
                            hprev[fc] = ht
                            # near the tail, out-DMA issue cost (~800ns serial
                            # per dma_start) dominates: spread across both rings
                            oe = out_eng
                            if pi >= len(pieces) - 2 and fc % 2 == 1:
                                oe = nc.scalar if out_eng is nc.sync else nc.sync
                            oe.dma_start(
                                outT[fc * FC:(fc + 1) * FC, poff_base:poff_base + pwidth],
                                ht[:],
                            )
                    off += width
                    boff += NKC * width
                prev_pwidth = pwidth

    nc.compile()
    return nc


def _to_in_dtype(a: np.ndarray) -> np.ndarray:
    if CONFIG["in_dtype"] == "bf16":
        import ml_dtypes
        return np.ascontiguousarray(a, dtype=np.float32).astype(ml_dtypes.bfloat16)
    return _round_fp32r(a)


def _prep_in_maps(x: np.ndarray, W_hg: np.ndarray):
    x = np.asarray(x, dtype=np.float32)
    W_hg = np.asarray(W_hg, dtype=np.float32)
    widths = CONFIG["widths"]

    # xQ [128, NKC*S]: per partition p, chunk sc holds xT[k*128+p, off:off+w]
    # k-major and contiguous, matching the SBUF tile image exactly.
    xQs = []
    for b in range(B):
        xT = np.ascontiguousarray(x[b].T)            # [D, S]
        xr = xT.reshape(NKC, KC, S)                   # [k, p, s]
        segs = []
        off = 0
        for w in widths:
            seg = xr[:, :, off:off + w]               # [k, p, w]
            segs.append(seg.transpose(1, 0, 2).reshape(KC, NKC * w))
            off += w
        xQs.append(_to_in_dtype(np.concatenate(segs, axis=1)))

    # wQ [128, NFC*NKC*256]: fc-major, then k, then 128 hidden + 128 gate cols
    wQs = []
    for c in range(2):
        wt = np.empty((D, NFC, 2 * FC), dtype=np.float32)
        for fc in range(NFC):
            rows_h = W_hg[c * DH + fc * FC:c * DH + (fc + 1) * FC]      # [FC, D]
            rows_g = W_hg[D + c * DH + fc * FC:D + c * DH + (fc + 1) * FC]
            wt[:, fc, 0:FC] = rows_h.T
            wt[:, fc, FC:2 * FC] = rows_g.T
        wr = wt.reshape(NKC, KC, NFC, 2 * FC)         # [k, p, fc, e]
        wq = wr.transpose(1, 2, 0, 3).reshape(KC, NFC * NKC * 2 * FC)
        wQs.append(_to_in_dtype(wq))
    return [{"xQ": xQs[core // 2], "wQ": wQs[core % 2]} for core in range(N_CORES)]


def _get_runner():
    """Build the Bass module once and cache a compiled jax callable for it.

    Mirrors bass2jax.run_bass_via_pjrt's multi-core path, but keeps the
    jitted/sharded executable so repeat kernel() calls skip re-tracing.
    """
    if "runner" in _CACHE:
        return _CACHE["runner"]

    import jax
    from jax.experimental.shard_map import shard_map
    from jax.sharding import Mesh, PartitionSpec
    from concourse import bass2jax

    if "nc" not in _CACHE:
        _CACHE["nc"] = _build()
    nc = _CACHE["nc"]
    bass2jax.install_neuronx_cc_hook()

    in_names = ["xQ", "wQ"]
    out_name = "outT"
    out_shape, out_dtype = (DH, S), np.float32
    partition_name = nc.partition_id_tensor.name if nc.partition_id_tensor else None

    def _body(xT, wT, zout):
        operands = [xT, wT, zout]
        if partition_name is not None:
            operands.append(bass2jax.partition_id_tensor())
        outs = bass2jax._bass_exec_p.bind(
            *operands,
            out_avals=(jax.core.ShapedArray(out_shape, out_dtype),),
            in_names=tuple(in_names + [out_name] + ([partition_name] if partition_name else [])),
            out_names=(out_name,),
            lowering_input_output_aliases=(),
            sim_require_finite=True,
            sim_require_nnan=True,
            nc=nc,
        )
        return tuple(outs)

    devices = jax.devices()[:N_CORES]
    mesh = Mesh(np.asarray(devices), ("core",))
    sharded = jax.jit(
        shard_map(
            _body, mesh=mesh,
            in_specs=(PartitionSpec("core"),) * 3,
            out_specs=(PartitionSpec("core"),),
            check_rep=False,
        ),
        donate_argnums=(2,),
        keep_unused=True,
    )

    def run(in_maps):
        concat_x = np.concatenate([m["xQ"] for m in in_maps], axis=0)
        concat_w = np.concatenate([m["wQ"] for m in in_maps], axis=0)
        zeros = np.zeros((N_CORES * DH, S), np.float32)
        (out_arr,) = sharded(concat_x, concat_w, zeros)
        return np.asarray(out_arr).reshape(N_CORES, DH, S)

    _CACHE["runner"] = run
    return run


def kernel(x: np.ndarray, W_hg: np.ndarray) -> np.ndarray:
    run = _get_runner()
    in_maps = _prep_in_maps(x, W_hg)
    outs = run(in_maps)

    out = np.empty((B, S, D), dtype=np.float32)
    for core in range(N_CORES):
        b, c = core // 2, core % 2
        out[b, :, c * DH:(c + 1) * DH] = outs[core].T
    return out


# revision 44
# speedup vs baseline: 1.0092x; 1.0059x over previous
"""MinGRU forward on 8 TRN2 NeuronCores.

Math (linear-space reformulation of the reference's log-space Heinsen scan):
    hg = x @ W_hg.T                       # [B,S,2D]
    hidden, gate = split(hg)
    z = sigmoid(gate)
    c = sigmoid(-gate)                    # = 1 - z = exp(-softplus(gate))
    g = max(hidden + 0.5, sigmoid(hidden))  # == where(h>=0, h+0.5, sigmoid(h)) exactly
    u = z * g
    h[t] = c[t] * h[t-1] + u[t]           # convex combination -> bounded, stable
    out = h

The recurrence maps directly onto the DVE `tensor_tensor_scan` instruction
(state = data0*state + data1 along the free dim, fp32 internal state).

Sharding: 8 cores = 4 batches x 2 feature-halves (512 features each).
No cross-core communication: the scan is per-feature independent.
The host packs x and W into the exact per-partition SBUF image (transposed,
k-major, chunk-contiguous) so every chunk is one DMA with multi-KB
contiguous lines.  Inputs are bf16 (halves DMA traffic, enables fast
weight load, rel-err ~1.5e-2 vs the 2e-2 gate); matmuls accumulate fp32
in PSUM; everything downstream of PSUM is fp32.
"""

import numpy as np

B, S, D = 4, 4096, 1024
DH = D // 2          # features per core
N_CORES = 8
KC = 128             # contraction chunk
NKC = D // KC        # 8 k chunks
FC = 128             # feature chunk (psum partitions)
NFC = DH // FC       # 4 feature chunks

_CACHE = {}

# build-time knobs (A/B tuning)
CONFIG = {
    "in_dtype": "bf16",   # "bf16" | "f32r"
    "out_ring": "sync",   # "scalar" | "sync"
    "widths": [512, 512, 512, 512, 512, 512, 512, 384, 128],
    "xbufs": 3,
    "psbufs": 4,
    "ebufs": 3,
    "u_on_gpsimd": False,  # compute u = z*g on GpSimd instead of DVE
    # (measured: GpSimd tensor ops contend for SBUF ports and slow PE/DVE/ACT
    # by ~13-20% across the board — keep ALL tensor work off GpSimd)
    "split_last_scan": False,  # last chunk: 2 chained half-scans (extra DMAs cost
    # ~830ns serial issue each at the tail — net loss)
    # The framework's end-of-kernel epilogue zeroes the ENTIRE kernel
    # semaphore range one EVENT_SEMAPHORE per sem (~28ns each, ~7us for the
    # default [7,256) range) inside the measured window.  The kernel needs
    # far fewer sems (the Tile allocator recycles via mid-kernel range
    # clears), so narrowing the range shrinks that epilogue directly.
    # 160 exhausts the pool; 184 works but measures the same as 216 (the
    # per-engine zeroing blocks run in parallel), so keep the extra margin.
    "sem_range_stop": 216,  # None = framework default (256)
}


def _round_fp32r(a: np.ndarray) -> np.ndarray:
    """Round fp32 array to fp32r (11 explicit mantissa bits) with RNE."""
    u = np.ascontiguousarray(a, dtype=np.float32).view(np.uint32)
    r = (u + np.uint32(0x7FF) + ((u >> np.uint32(12)) & np.uint32(1))) & np.uint32(0xFFFFF000)
    return r.view(np.float32)


def _build():
    import concourse.bacc as bacc
    import concourse.bass as bass_mod
    import concourse.tile as tile
    import concourse.mybir as mybir

    if CONFIG["sem_range_stop"] is not None:
        stop = CONFIG["sem_range_stop"]
        bass_mod.get_kernel_semaphore_range = (
            lambda: range(bass_mod.get_walrus_max_sem_num(), stop)
        )

    f32 = mybir.dt.float32
    in_dt = mybir.dt.bfloat16 if CONFIG["in_dtype"] == "bf16" else mybir.dt.float32r
    AF = mybir.ActivationFunctionType
    OP = mybir.AluOpType

    nc = bacc.Bacc("TRN2")
    # Host pre-packs both inputs into the exact SBUF image the kernel wants:
    # per partition p, chunk sc occupies a contiguous [NKC * width] run
    # (k-major).  This makes every chunk ONE DMA with multi-KB contiguous
    # lines per partition — short lines (<2KB) tank HWDGE throughput.
    xQ = nc.dram_tensor("xQ", [KC, NKC * S], in_dt, kind="ExternalInput")
    # wQ: per partition, fc-major then k-major then 256 cols (128 hidden,
    # 128 gate).
    wQ = nc.dram_tensor("wQ", [KC, NFC * NKC * 2 * FC], in_dt, kind="ExternalInput")
    outT = nc.dram_tensor("outT", [DH, S], f32, kind="ExternalOutput")

    widths = CONFIG["widths"]
    assert sum(widths) == S
    WFC = NKC * 2 * FC  # weight elems per partition per fc (2048)

    with tile.TileContext(nc) as tc:
        with (
            tc.tile_pool(name="w", bufs=1) as wpool,
            tc.tile_pool(name="x", bufs=CONFIG["xbufs"]) as xpool,
            tc.tile_pool(name="ew", bufs=CONFIG["ebufs"]) as epool,
            tc.tile_pool(name="h", bufs=2) as hpool,
            tc.tile_pool(name="ps", bufs=CONFIG["psbufs"], space="PSUM") as pspool,
        ):
            # W rides the ACT HWDGE ring; x rides the SP ring.  The ramp is
            # DMA-critical: the very first descriptors on each ring are the
            # small slices the first matmuls need, so the PE starts early
            # and stays busy (warming HAM's clock-gate with real work).
            wts = []
            for fc in range(NFC):
                wtf = wpool.tile([KC, WFC], in_dt, tag=f"w{fc}")
                wts.append(wtf)

            # (HAM clock-gate warm-up via dummy matmuls was tried twice —
            # sparse tiny MMs don't trip the activity window, and dense ones
            # reshuffle the Tile scheduler's DMA issue order and starve the
            # early pipeline.  Both regressed; do not re-add.)

            # fc0 weights and x chunk 0 each arrive in four k-pair slices so
            # the first matmuls start (and keep going) as the data streams in
            # — paced PE consumption also warms the HAM clock-gate sooner.
            for q in range(4):
                a, b = q * WFC // 4, (q + 1) * WFC // 4
                nc.scalar.dma_start(wts[0][:, a:b], wQ[:, a:b])
            w0 = widths[0]
            xt0 = xpool.tile([KC, NKC * w0], in_dt, tag="xt")
            # k-pair slices (NOT k0-alone-first: starting the PE earlier on a
            # sparser trickle delays the HAM busy-window trip and nets worse)
            for q in range(4):
                a, b = q * 2 * w0, (q + 1) * 2 * w0
                nc.sync.dma_start(xt0[:, a:b], xQ[:, a:b])
            # Chunk 1's first half rides the SCALAR ring between wfc2 and
            # wfc3 (W has slack there): on the sync ring behind all of x0 it
            # lands right at the PE's need time (~16us), and that marginal
            # wait is what occasionally crosses a HAM MID window and
            # re-throttles the clock mid-ramp.
            w1 = widths[1]
            xt1 = xpool.tile([KC, NKC * w1], in_dt, tag="xt", name="xt1")
            x1off = NKC * widths[0]
            h1_ = NKC * w1 // 2
            nc.scalar.dma_start(wts[1][:], wQ[:, WFC:2 * WFC])
            nc.scalar.dma_start(wts[2][:], wQ[:, 2 * WFC:3 * WFC])
            nc.scalar.dma_start(xt1[:, 0:h1_], xQ[:, x1off:x1off + h1_])
            nc.scalar.dma_start(wts[3][:], wQ[:, 3 * WFC:4 * WFC])
            nc.sync.dma_start(xt1[:, h1_:], xQ[:, x1off + h1_:x1off + NKC * w1])

            out_eng = nc.scalar if CONFIG["out_ring"] == "scalar" else nc.sync

            # One piece (scan + out-DMA granule) per chunk.  Pairing chunks
            # into wider pieces was measured neutral-to-worse: the PE, not
            # the DVE, is the binding engine, and wide pieces near the end
            # push large output transfers into the tail.
            pieces = [(sc,) for sc in range(len(widths))]

            hprev = [None] * NFC
            prev_pwidth = 0
            off = 0
            boff = 0  # element offset into xQ's per-partition run
            for pi, piece in enumerate(pieces):
                pwidth = sum(widths[s] for s in piece)
                poff_base = off
                cts = [None] * NFC
                uts = [None] * NFC
                for j, sc in enumerate(piece):
                    width = widths[sc]
                    if sc == 0:
                        xt = xt0
                    elif sc == 1:
                        xt = xt1  # prefetched above, split across both rings
                    else:
                        xt = xpool.tile([KC, NKC * width], in_dt, tag="xt")
                        nc.sync.dma_start(xt[:], xQ[:, boff:boff + NKC * width])
                    poff = off - poff_base
                    for fc in range(NFC):
                        ph = pspool.tile([FC, width], f32, tag="ph")
                        pg = pspool.tile([FC, width], f32, tag="pg")
                        if sc == 0:
                            # ramp: alternate h/g per k so the PE consumes
                            # each k-pair DMA slice the moment it lands
                            for k in range(NKC):
                                nc.tensor.matmul(
                                    ph[:], wts[fc][:, k * 2 * FC:k * 2 * FC + FC],
                                    xt[:, k * width:(k + 1) * width],
                                    start=(k == 0), stop=(k == NKC - 1),
                                )
                                nc.tensor.matmul(
                                    pg[:], wts[fc][:, k * 2 * FC + FC:(k + 1) * 2 * FC],
                                    xt[:, k * width:(k + 1) * width],
                                    start=(k == 0), stop=(k == NKC - 1),
                                )
                        else:
                            for k in range(NKC):
                                nc.tensor.matmul(
                                    ph[:], wts[fc][:, k * 2 * FC:k * 2 * FC + FC],
                                    xt[:, k * width:(k + 1) * width],
                                    start=(k == 0), stop=(k == NKC - 1),
                                )
                            for k in range(NKC):
                                nc.tensor.matmul(
                                    pg[:], wts[fc][:, k * 2 * FC + FC:(k + 1) * 2 * FC],
                                    xt[:, k * width:(k + 1) * width],
                                    start=(k == 0), stop=(k == NKC - 1),
                                )
                        if j == 0:
                            cts[fc] = hpool.tile(
                                [FC, pwidth], f32, tag=f"c{fc}", name=f"c{fc}"
                            )
                            uts[fc] = hpool.tile(
                                [FC, pwidth], f32, tag=f"u{fc}", name=f"u{fc}"
                            )
                        ct, ut = cts[fc], uts[fc]
                        zt = epool.tile([FC, width], f32, tag="z")
                        st = epool.tile([FC, width], f32, tag="s")
                        gt = epool.tile([FC, width], f32, tag="g")
                        # s first: it heads the DVE critical chain (s->g->u->scan)
                        nc.scalar.activation(st[:], ph[:], AF.Sigmoid)
                        nc.scalar.activation(zt[:], pg[:], AF.Sigmoid)
                        # c = 1 - z on the DVE (fast 2x tensor_scalar) instead
                        # of a third ACT sigmoid: ACT is the tail serializer
                        nc.vector.tensor_scalar(
                            ct[:, poff:poff + width], zt[:], -1.0, 1.0,
                            op0=OP.mult, op1=OP.add,
                        )
                        # g = (hidden + 0.5) max sigmoid(hidden)
                        nc.vector.scalar_tensor_tensor(
                            gt[:], ph[:], 0.5, st[:], op0=OP.add, op1=OP.max
                        )
                        nc.vector.tensor_mul(ut[:, poff:poff + width], zt[:], gt[:])
                        if j == len(piece) - 1:
                            ht = hpool.tile([FC, pwidth], f32, tag=f"h{fc}")
                            init = (
                                0.0 if pi == 0
                                else hprev[fc][:, prev_pwidth - 1:prev_pwidth]
                            )
                            nc.vector.tensor_tensor_scan(
                                ht[:], ct[:], ut[:], init, op0=OP.mult, op1=OP.add
                            )
                            hprev[fc] = ht
                            # near the tail, out-DMA issue cost (~800ns serial
                            # per dma_start) dominates: spread across both rings
                            oe = out_eng
                            if pi >= len(pieces) - 2 and fc % 2 == 1:
                                oe = nc.scalar if out_eng is nc.sync else nc.sync
                            oe.dma_start(
                                outT[fc * FC:(fc + 1) * FC, poff_base:poff_base + pwidth],
                                ht[:],
                            )
                    off += width
                    boff += NKC * width
                prev_pwidth = pwidth

    nc.compile()
    return nc


def _to_in_dtype(a: np.ndarray) -> np.ndarray:
    if CONFIG["in_dtype"] == "bf16":
        import ml_dtypes
        return np.ascontiguousarray(a, dtype=np.float32).astype(ml_dtypes.bfloat16)
    return _round_fp32r(a)


def _prep_in_maps(x: np.ndarray, W_hg: np.ndarray):
    x = np.asarray(x, dtype=np.float32)
    W_hg = np.asarray(W_hg, dtype=np.float32)
    widths = CONFIG["widths"]

    # xQ [128, NKC*S]: per partition p, chunk sc holds xT[k*128+p, off:off+w]
    # k-major and contiguous, matching the SBUF tile image exactly.
    xQs = []
    for b in range(B):
        xT = np.ascontiguousarray(x[b].T)            # [D, S]
        xr = xT.reshape(NKC, KC, S)                   # [k, p, s]
        segs = []
        off = 0
        for w in widths:
            seg = xr[:, :, off:off + w]               # [k, p, w]
            segs.append(seg.transpose(1, 0, 2).reshape(KC, NKC * w))
            off += w
        xQs.append(_to_in_dtype(np.concatenate(segs, axis=1)))

    # wQ [128, NFC*NKC*256]: fc-major, then k, then 128 hidden + 128 gate cols
    wQs = []
    for c in range(2):
        wt = np.empty((D, NFC, 2 * FC), dtype=np.float32)
        for fc in range(NFC):
            rows_h = W_hg[c * DH + fc * FC:c * DH + (fc + 1) * FC]      # [FC, D]
            rows_g = W_hg[D + c * DH + fc * FC:D + c * DH + (fc + 1) * FC]
            wt[:, fc, 0:FC] = rows_h.T
            wt[:, fc, FC:2 * FC] = rows_g.T
        wr = wt.reshape(NKC, KC, NFC, 2 * FC)         # [k, p, fc, e]
        wq = wr.transpose(1, 2, 0, 3).reshape(KC, NFC * NKC * 2 * FC)
        wQs.append(_to_in_dtype(wq))
    return [{"xQ": xQs[core // 2], "wQ": wQs[core % 2]} for core in range(N_CORES)]


def _get_runner():
    """Build the Bass module once and cache a compiled jax callable for it.

    Mirrors bass2jax.run_bass_via_pjrt's multi-core path, but keeps the
    jitted/sharded executable so repeat kernel() calls skip re-tracing.
    """
    if "runner" in _CACHE:
        return _CACHE["runner"]

    import jax
    from jax.experimental.shard_map import shard_map
    from jax.sharding import Mesh, PartitionSpec
    from concourse import bass2jax

    if "nc" not in _CACHE:
        _CACHE["nc"] = _build()
    nc = _CACHE["nc"]
    bass2jax.install_neuronx_cc_hook()

    in_names = ["xQ", "wQ"]
    out_name = "outT"
    out_shape, out_dtype = (DH, S), np.float32
    partition_name = nc.partition_id_tensor.name if nc.partition_id_tensor else None

    def _body(xT, wT, zout):
        operands = [xT, wT, zout]
        if partition_name is not None:
            operands.append(bass2jax.partition_id_tensor())
        outs = bass2jax._bass_exec_p.bind(
            *operands,
            out_avals=(jax.core.ShapedArray(out_shape, out_dtype),),
            in_names=tuple(in_names + [out_name] + ([partition_name] if partition_name else [])),
            out_names=(out_name,),
            lowering_input_output_aliases=(),
            sim_require_finite=True,
            sim_require_nnan=True,
            nc=nc,
        )
        return tuple(outs)

    devices = jax.devices()[:N_CORES]
    mesh = Mesh(np.asarray(devices), ("core",))
    sharded = jax.jit(
        shard_map(
            _body, mesh=mesh,
            in_specs=(PartitionSpec("core"),) * 3,
            out_specs=(PartitionSpec("core"),),
            check_rep=False,
        ),
        donate_argnums=(2,),
        keep_unused=True,
    )

    def run(in_maps):
        concat_x = np.concatenate([m["xQ"] for m in in_maps], axis=0)
        concat_w = np.concatenate([m["wQ"] for m in in_maps], axis=0)
        zeros = np.zeros((N_CORES * DH, S), np.float32)
        (out_arr,) = sharded(concat_x, concat_w, zeros)
        return np.asarray(out_arr).reshape(N_CORES, DH, S)

    _CACHE["runner"] = run
    return run


def kernel(x: np.ndarray, W_hg: np.ndarray) -> np.ndarray:
    run = _get_runner()
    in_maps = _prep_in_maps(x, W_hg)
    outs = run(in_maps)

    out = np.empty((B, S, D), dtype=np.float32)
    for core in range(N_CORES):
        b, c = core // 2, core % 2
        out[b, :, c * DH:(c + 1) * DH] = outs[core].T
    return out


# revision 46
# speedup vs baseline: 1.0243x; 1.0149x over previous
"""MinGRU forward on 8 TRN2 NeuronCores.

Math (linear-space reformulation of the reference's log-space Heinsen scan):
    hg = x @ W_hg.T                       # [B,S,2D]
    hidden, gate = split(hg)
    z = sigmoid(gate)
    c = sigmoid(-gate)                    # = 1 - z = exp(-softplus(gate))
    g = max(hidden + 0.5, sigmoid(hidden))  # == where(h>=0, h+0.5, sigmoid(h)) exactly
    u = z * g
    h[t] = c[t] * h[t-1] + u[t]           # convex combination -> bounded, stable
    out = h

The recurrence maps directly onto the DVE `tensor_tensor_scan` instruction
(state = data0*state + data1 along the free dim, fp32 internal state).

Sharding: 8 cores = 4 batches x 2 feature-halves (512 features each).
No cross-core communication: the scan is per-feature independent.
The host packs x and W into the exact per-partition SBUF image (transposed,
k-major, chunk-contiguous) so every chunk is one DMA with multi-KB
contiguous lines.  Inputs are bf16 (halves DMA traffic, enables fast
weight load, rel-err ~1.5e-2 vs the 2e-2 gate); matmuls accumulate fp32
in PSUM; everything downstream of PSUM is fp32.
"""

import numpy as np

B, S, D = 4, 4096, 1024
DH = D // 2          # features per core
N_CORES = 8
KC = 128             # contraction chunk
NKC = D // KC        # 8 k chunks
FC = 128             # feature chunk (psum partitions)
NFC = DH // FC       # 4 feature chunks

_CACHE = {}

# build-time knobs (A/B tuning)
CONFIG = {
    "in_dtype": "bf16",   # "bf16" | "f32r"
    "out_ring": "sync",   # "scalar" | "sync"
    "widths": [512, 512, 512, 512, 512, 512, 512, 384, 128],
    "xbufs": 3,
    "psbufs": 4,
    "ebufs": 3,
    "u_on_gpsimd": False,  # compute u = z*g on GpSimd instead of DVE
    # (measured: GpSimd tensor ops contend for SBUF ports and slow PE/DVE/ACT
    # by ~13-20% across the board — keep ALL tensor work off GpSimd)
    "split_last_scan": False,  # last chunk: 2 chained half-scans (extra DMAs cost
    # ~830ns serial issue each at the tail — net loss)
    # The framework's end-of-kernel epilogue zeroes the ENTIRE kernel
    # semaphore range one EVENT_SEMAPHORE per sem (~28ns each, ~7us for the
    # default [7,256) range) inside the measured window.  The kernel needs
    # far fewer sems (the Tile allocator recycles via mid-kernel range
    # clears), so narrowing the range shrinks that epilogue directly.
    # 160 exhausts the pool; 184 works but measures the same as 216 (the
    # per-engine zeroing blocks run in parallel), so keep the extra margin.
    "sem_range_stop": 216,  # None = framework default (256)
}


def _round_fp32r(a: np.ndarray) -> np.ndarray:
    """Round fp32 array to fp32r (11 explicit mantissa bits) with RNE."""
    u = np.ascontiguousarray(a, dtype=np.float32).view(np.uint32)
    r = (u + np.uint32(0x7FF) + ((u >> np.uint32(12)) & np.uint32(1))) & np.uint32(0xFFFFF000)
    return r.view(np.float32)


def _build():
    import concourse.bacc as bacc
    import concourse.bass as bass_mod
    import concourse.tile as tile
    import concourse.mybir as mybir

    if CONFIG["sem_range_stop"] is not None:
        stop = CONFIG["sem_range_stop"]
        bass_mod.get_kernel_semaphore_range = (
            lambda: range(bass_mod.get_walrus_max_sem_num(), stop)
        )

    f32 = mybir.dt.float32
    in_dt = mybir.dt.bfloat16 if CONFIG["in_dtype"] == "bf16" else mybir.dt.float32r
    AF = mybir.ActivationFunctionType
    OP = mybir.AluOpType

    nc = bacc.Bacc("TRN2")
    # Host pre-packs both inputs into the exact SBUF image the kernel wants:
    # per partition p, chunk sc occupies a contiguous [NKC * width] run
    # (k-major).  This makes every chunk ONE DMA with multi-KB contiguous
    # lines per partition — short lines (<2KB) tank HWDGE throughput.
    xQ = nc.dram_tensor("xQ", [KC, NKC * S], in_dt, kind="ExternalInput")
    # wQ: per partition, fc-major then k-major then 256 cols (128 hidden,
    # 128 gate).
    wQ = nc.dram_tensor("wQ", [KC, NFC * NKC * 2 * FC], in_dt, kind="ExternalInput")
    outT = nc.dram_tensor("outT", [DH, S], f32, kind="ExternalOutput")

    widths = CONFIG["widths"]
    assert sum(widths) == S
    WFC = NKC * 2 * FC  # weight elems per partition per fc (2048)

    with tile.TileContext(nc) as tc:
        with (
            tc.tile_pool(name="w", bufs=1) as wpool,
            tc.tile_pool(name="x", bufs=CONFIG["xbufs"]) as xpool,
            tc.tile_pool(name="ew", bufs=CONFIG["ebufs"]) as epool,
            tc.tile_pool(name="h", bufs=2) as hpool,
            tc.tile_pool(name="ps", bufs=CONFIG["psbufs"], space="PSUM") as pspool,
        ):
            # W rides the ACT HWDGE ring; x rides the SP ring.  The ramp is
            # DMA-critical: the very first descriptors on each ring are the
            # small slices the first matmuls need, so the PE starts early
            # and stays busy (warming HAM's clock-gate with real work).
            wts = []
            for fc in range(NFC):
                wtf = wpool.tile([KC, WFC], in_dt, tag=f"w{fc}")
                wts.append(wtf)

            # (HAM clock-gate warm-up via dummy matmuls was tried twice —
            # sparse tiny MMs don't trip the activity window, and dense ones
            # reshuffle the Tile scheduler's DMA issue order and starve the
            # early pipeline.  Both regressed; do not re-add.)

            # fc0 weights and x chunk 0 each arrive in four k-pair slices so
            # the first matmuls start (and keep going) as the data streams in
            # — paced PE consumption also warms the HAM clock-gate sooner.
            for q in range(4):
                a, b = q * WFC // 4, (q + 1) * WFC // 4
                nc.scalar.dma_start(wts[0][:, a:b], wQ[:, a:b])
            w0 = widths[0]
            xt0 = xpool.tile([KC, NKC * w0], in_dt, tag="xt")
            # k-pair slices (NOT k0-alone-first: starting the PE earlier on a
            # sparser trickle delays the HAM busy-window trip and nets worse)
            for q in range(4):
                a, b = q * 2 * w0, (q + 1) * 2 * w0
                nc.sync.dma_start(xt0[:, a:b], xQ[:, a:b])
            for fc in range(1, NFC):
                nc.scalar.dma_start(wts[fc][:], wQ[:, fc * WFC:(fc + 1) * WFC])

            out_eng = nc.scalar if CONFIG["out_ring"] == "scalar" else nc.sync

            # One piece (scan + out-DMA granule) per chunk.  Pairing chunks
            # into wider pieces was measured neutral-to-worse: the PE, not
            # the DVE, is the binding engine, and wide pieces near the end
            # push large output transfers into the tail.
            pieces = [(sc,) for sc in range(len(widths))]

            hprev = [None] * NFC
            prev_pwidth = 0
            off = 0
            boff = 0  # element offset into xQ's per-partition run
            for pi, piece in enumerate(pieces):
                pwidth = sum(widths[s] for s in piece)
                poff_base = off
                cts = [None] * NFC
                uts = [None] * NFC
                for j, sc in enumerate(piece):
                    width = widths[sc]
                    if sc == 0:
                        xt = xt0
                    else:
                        xt = xpool.tile([KC, NKC * width], in_dt, tag="xt")
                        if sc == 1:
                            # still inside the ramp: split for earlier arrival
                            h_ = NKC * width // 2
                            nc.sync.dma_start(xt[:, 0:h_], xQ[:, boff:boff + h_])
                            nc.sync.dma_start(xt[:, h_:], xQ[:, boff + h_:boff + NKC * width])
                        else:
                            nc.sync.dma_start(xt[:], xQ[:, boff:boff + NKC * width])
                    poff = off - poff_base
                    for fc in range(NFC):
                        ph = pspool.tile([FC, width], f32, tag="ph")
                        pg = pspool.tile([FC, width], f32, tag="pg")
                        if sc == 0:
                            # ramp: alternate h/g per k so the PE consumes
                            # each k-pair DMA slice the moment it lands
                            for k in range(NKC):
                                nc.tensor.matmul(
                                    ph[:], wts[fc][:, k * 2 * FC:k * 2 * FC + FC],
                                    xt[:, k * width:(k + 1) * width],
                                    start=(k == 0), stop=(k == NKC - 1),
                                )
                                nc.tensor.matmul(
                                    pg[:], wts[fc][:, k * 2 * FC + FC:(k + 1) * 2 * FC],
                                    xt[:, k * width:(k + 1) * width],
                                    start=(k == 0), stop=(k == NKC - 1),
                                )
                        else:
                            for k in range(NKC):
                                nc.tensor.matmul(
                                    ph[:], wts[fc][:, k * 2 * FC:k * 2 * FC + FC],
                                    xt[:, k * width:(k + 1) * width],
                                    start=(k == 0), stop=(k == NKC - 1),
                                )
                            for k in range(NKC):
                                nc.tensor.matmul(
                                    pg[:], wts[fc][:, k * 2 * FC + FC:(k + 1) * 2 * FC],
                                    xt[:, k * width:(k + 1) * width],
                                    start=(k == 0), stop=(k == NKC - 1),
                                )
                        if j == 0:
                            cts[fc] = hpool.tile(
                                [FC, pwidth], f32, tag=f"c{fc}", name=f"c{fc}"
                            )
                            uts[fc] = hpool.tile(
                                [FC, pwidth], f32, tag=f"u{fc}", name=f"u{fc}"
                            )
                        ct, ut = cts[fc], uts[fc]
                        zt = epool.tile([FC, width], f32, tag="z")
                        st = epool.tile([FC, width], f32, tag="s")
                        gt = epool.tile([FC, width], f32, tag="g")
                        # s first: it heads the DVE critical chain (s->g->u->scan)
                        nc.scalar.activation(st[:], ph[:], AF.Sigmoid)
                        nc.scalar.activation(zt[:], pg[:], AF.Sigmoid)
                        # c = 1 - z on the DVE (fast 2x tensor_scalar) instead
                        # of a third ACT sigmoid: ACT is the tail serializer
                        nc.vector.tensor_scalar(
                            ct[:, poff:poff + width], zt[:], -1.0, 1.0,
                            op0=OP.mult, op1=OP.add,
                        )
                        # g = (hidden + 0.5) max sigmoid(hidden)
                        nc.vector.scalar_tensor_tensor(
                            gt[:], ph[:], 0.5, st[:], op0=OP.add, op1=OP.max
                        )
                        nc.vector.tensor_mul(ut[:, poff:poff + width], zt[:], gt[:])
                        if j == len(piece) - 1:
                            ht = hpool.tile([FC, pwidth], f32, tag=f"h{fc}")
                            init = (
                                0.0 if pi == 0
                                else hprev[fc][:, prev_pwidth - 1:prev_pwidth]
                            )
                            nc.vector.tensor_tensor_scan(
                                ht[:], ct[:], ut[:], init, op0=OP.mult, op1=OP.add
                            )
                            hprev[fc] = ht
                            # near the tail, out-DMA issue cost (~800ns serial
                            # per dma_start) dominates: spread across both rings
                            oe = out_eng
                            if pi >= len(pieces) - 2 and fc % 2 == 1:
                                oe = nc.scalar if out_eng is nc.sync else nc.sync
                            oe.dma_start(
                                outT[fc * FC:(fc + 1) * FC, poff_base:poff_base + pwidth],
                                ht[:],
                            )
                    off += width
                    boff += NKC * width
                prev_pwidth = pwidth

    nc.compile()
    return nc


def _to_in_dtype(a: np.ndarray) -> np.ndarray:
    if CONFIG["in_dtype"] == "bf16":
        import ml_dtypes
        return np.ascontiguousarray(a, dtype=np.float32).astype(ml_dtypes.bfloat16)
    return _round_fp32r(a)


def _prep_in_maps(x: np.ndarray, W_hg: np.ndarray):
    x = np.asarray(x, dtype=np.float32)
    W_hg = np.asarray(W_hg, dtype=np.float32)
    widths = CONFIG["widths"]

    # xQ [128, NKC*S]: per partition p, chunk sc holds xT[k*128+p, off:off+w]
    # k-major and contiguous, matching the SBUF tile image exactly.
    xQs = []
    for b in range(B):
        xT = np.ascontiguousarray(x[b].T)            # [D, S]
        xr = xT.reshape(NKC, KC, S)                   # [k, p, s]
        segs = []
        off = 0
        for w in widths:
            seg = xr[:, :, off:off + w]               # [k, p, w]
            segs.append(seg.transpose(1, 0, 2).reshape(KC, NKC * w))
            off += w
        xQs.append(_to_in_dtype(np.concatenate(segs, axis=1)))

    # wQ [128, NFC*NKC*256]: fc-major, then k, then 128 hidden + 128 gate cols
    wQs = []
    for c in range(2):
        wt = np.empty((D, NFC, 2 * FC), dtype=np.float32)
        for fc in range(NFC):
            rows_h = W_hg[c * DH + fc * FC:c * DH + (fc + 1) * FC]      # [FC, D]
            rows_g = W_hg[D + c * DH + fc * FC:D + c * DH + (fc + 1) * FC]
            wt[:, fc, 0:FC] = rows_h.T
            wt[:, fc, FC:2 * FC] = rows_g.T
        wr = wt.reshape(NKC, KC, NFC, 2 * FC)         # [k, p, fc, e]
        wq = wr.transpose(1, 2, 0, 3).reshape(KC, NFC * NKC * 2 * FC)
        wQs.append(_to_in_dtype(wq))
    return [{"xQ": xQs[core // 2], "wQ": wQs[core % 2]} for core in range(N_CORES)]


def _get_runner():
    """Build the Bass module once and cache a compiled jax callable for it.

    Mirrors bass2jax.run_bass_via_pjrt's multi-core path, but keeps the
    jitted/sharded executable so repeat kernel() calls skip re-tracing.
    """
    if "runner" in _CACHE:
        return _CACHE["runner"]

    import jax
    from jax.experimental.shard_map import shard_map
    from jax.sharding import Mesh, PartitionSpec
    from concourse import bass2jax

    if "nc" not in _CACHE:
        _CACHE["nc"] = _build()
    nc = _CACHE["nc"]
    bass2jax.install_neuronx_cc_hook()

    in_names = ["xQ", "wQ"]
    out_name = "outT"
    out_shape, out_dtype = (DH, S), np.float32
    partition_name = nc.partition_id_tensor.name if nc.partition_id_tensor else None

    def _body(xT, wT, zout):
        operands = [xT, wT, zout]
        if partition_name is not None:
            operands.append(bass2jax.partition_id_tensor())
        outs = bass2jax._bass_exec_p.bind(
            *operands,
            out_avals=(jax.core.ShapedArray(out_shape, out_dtype),),
            in_names=tuple(in_names + [out_name] + ([partition_name] if partition_name else [])),
            out_names=(out_name,),
            lowering_input_output_aliases=(),
            sim_require_finite=True,
            sim_require_nnan=True,
            nc=nc,
        )
        return tuple(outs)

    devices = jax.devices()[:N_CORES]
    mesh = Mesh(np.asarray(devices), ("core",))
    sharded = jax.jit(
        shard_map(
            _body, mesh=mesh,
            in_specs=(PartitionSpec("core"),) * 3,
            out_specs=(PartitionSpec("core"),),
            check_rep=False,
        ),
        donate_argnums=(2,),
        keep_unused=True,
    )

    def run(in_maps):
        concat_x = np.concatenate([m["xQ"] for m in in_maps], axis=0)
        concat_w = np.concatenate([m["wQ"] for m in in_maps], axis=0)
        zeros = np.zeros((N_CORES * DH, S), np.float32)
        (out_arr,) = sharded(concat_x, concat_w, zeros)
        return np.asarray(out_arr).reshape(N_CORES, DH, S)

    _CACHE["runner"] = run
    return run


def kernel(x: np.ndarray, W_hg: np.ndarray) -> np.ndarray:
    run = _get_runner()
    in_maps = _prep_in_maps(x, W_hg)
    outs = run(in_maps)

    out = np.empty((B, S, D), dtype=np.float32)
    for core in range(N_CORES):
        b, c = core // 2, core % 2
        out[b, :, c * DH:(c + 1) * DH] = outs[core].T
    return out
